# revision 25
# baseline (speedup 1.0000x reference)
"""Trainium2 Bass kernel for nn_InterpNetwork (gnn_message_passing).

Sharding: 2 graphs x 32 transitions = 64 (g,t) units -> 8 cores get 8
transitions each (cores 0-3: graph 0, cores 4-7: graph 1). Weights
replicated; no collectives.

Per-core device pipeline (feature-major layouts, features on partitions):
  - node encoder: 2 matmuls over all 9 local frames at once + ACT relu.
  - edge encoder: K=1 outer-product matmul + K=32 matmul over the 9216
    (i,j) pairs, relu split between ACT and DVE.
  - pairwise update-detector layer 1 decomposed into 3 accumulating
    matmuls per 384-pair chunk: xne[:,i] / xce[:,j] broadcast via
    stride-0 access patterns on the moving operand (K=128 each) plus the
    edge-feature pass (K=32).  ACT applies bias+relu from PSUM.
  - layer 2 (128->64) matmul + DVE fused bias+relu; layer 3 (64->1)
    matmul; gpsimd SWDGE scatters the (1,384) rows into a (96,96) tile,
    DVE applies the precomputed edge masks (bias+(-1e9) fill), DMA out.
  - dist head batched over all 8 transitions at the end.
Matmuls run as float32r (1 cyc/row at N>=256) with fp32 accumulate.
"""

import ml_dtypes
import numpy as np

import concourse.bass as bass
from concourse import bacc
import concourse.mybir as mybir
import concourse.tile as tile
from concourse.bass_utils import run_bass_kernel_spmd

F32 = mybir.dt.float32
BF16 = mybir.dt.bfloat16
RELU = mybir.ActivationFunctionType.Relu
ADD = mybir.AluOpType.add
MAX = mybir.AluOpType.max

G, T, H, D = 2, 33, 128, 96
NT = 8            # transitions per core
NF = NT + 1       # frames per core
NPAIR = D * D     # 9216
NEG = -1e9

# packed-weight layouts: (name, rows, cols) -> column offsets accumulate
PACK_F32 = [
    ("neW1", 128, 128), ("neW2", 128, 128), ("dpW1c", 128, 128),
    ("dpW1n", 128, 128), ("dpW2", 128, 64), ("neb1", 128, 1),
    ("neb2", 128, 1), ("udb1", 128, 1), ("udb2s", 128, 1), ("dpb1", 128, 1),
    ("dpb2", 64, 1), ("dpW3", 64, 1), ("dpb3", 1, 1), ("eeb1", 32, 1),
    ("eeb2", 32, 1), ("addm2", 2, NPAIR // 2),
]
PACK_BF16 = [
    ("udW1n", 128, 128), ("udW1c", 128, 128), ("udW1e", 32, 128),
    ("udW2", 128, 64), ("udW3d", 128, 2), ("eeW1", 1, 32), ("eeW2", 32, 32),
    ("ewf", 1, NPAIR),
]
NCOL_F32 = sum(c for _, _, c in PACK_F32)
NCOL_BF16 = sum(c for _, _, c in PACK_BF16)

_prog_cache = {}


def _w3_blockdiag(w3):
    w3 = np.asarray(w3, np.float32).reshape(64)
    out = np.zeros((128, 2), np.float32)
    out[0:64, 0] = w3
    out[64:128, 1] = w3
    return out


def _build_program():
    nc = bacc.Bacc()

    def din(name, shape, dt=F32):
        return nc.declare_dram_parameter(name, list(shape), dt, isOutput=False)

    xin = din("xin", (H, NF * D))          # 9 frames feature-major
    wpk = din("wpk", (128, NCOL_F32))
    wpkb = din("wpkb", (128, NCOL_BF16), BF16)

    cls = nc.declare_dram_parameter("cls", [NT, D, D], F32, isOutput=True)
    dst = nc.declare_dram_parameter("dst", [1, NT * D], F32, isOutput=True)

    with tile.TileContext(nc) as tc:
        with (
            tc.tile_pool(name="w", bufs=1) as wp,
            tc.tile_pool(name="big", bufs=1) as bp,
            tc.tile_pool(name="h1", bufs=3) as h1p,
            tc.tile_pool(name="h2", bufs=3) as h2p,
            tc.tile_pool(name="st", bufs=2) as stp,
            tc.tile_pool(name="ps1", bufs=2, space="PSUM") as ps1p,
            tc.tile_pool(name="ps2", bufs=1, space="PSUM") as ps2p,
            tc.tile_pool(name="ps3", bufs=2, space="PSUM") as ps3p,
        ):
            wsb = wp.tile([128, NCOL_F32], F32, tag="wpk")
            nc.gpsimd.dma_start(out=wsb[:, :], in_=wpk[:, :])
            wsbb = wp.tile([128, NCOL_BF16], BF16, tag="wpkb")
            nc.gpsimd.dma_start(out=wsbb[:, :], in_=wpkb[:, :])
            w = {}
            off = 0
            for nm, nr, ncol in PACK_F32:
                w[nm] = wsb[0:nr, off:off + ncol]
                off += ncol
            off = 0
            for nm, nr, ncol in PACK_BF16:
                w[nm] = wsbb[0:nr, off:off + ncol]
                off += ncol

            def wr(name):
                return w[name]

            # ---- node encoder over all NF frames ----
            x_sb = bp.tile([H, NF * D], F32, tag="x")
            nc.gpsimd.dma_start(out=x_sb[:, :], in_=xin[:, :])
            xe1 = bp.tile([128, NF * D], F32, tag="xe1")
            NE_CH = 432  # 864 = 2 chunks, each >=256 so f32r runs 1cyc/row
            for c in range(2):
                sl = slice(c * NE_CH, (c + 1) * NE_CH)
                ps = ps1p.tile([128, NE_CH], F32, tag="ps1")
                nc.tensor.matmul(ps[:, :], wr("neW1"),
                                 x_sb[:, sl], start=True, stop=True)
                nc.scalar.activation(xe1[:, sl], ps[:, :], RELU, bias=w["neb1"])
            xe = bp.tile([128, NF * D], F32, tag="xe")
            xe_bf = bp.tile([128, NF * D], BF16, tag="xebf")
            for c in range(2):
                sl = slice(c * NE_CH, (c + 1) * NE_CH)
                ps = ps1p.tile([128, NE_CH], F32, tag="ps1")
                nc.tensor.matmul(ps[:, :], wr("neW2"),
                                 xe1[:, sl], start=True, stop=True)
                nc.scalar.activation(xe[:, sl], ps[:, :], RELU, bias=w["neb2"])
                nc.scalar.activation(xe_bf[:, sl], ps[:, :], RELU,
                                     bias=w["neb2"])
            xer = xe_bf[:, :]

            # ---- edge encoder over all 9216 pairs ----
            ew_sb = w["ewf"]
            e1r = bp.tile([32, NPAIR], BF16, tag="e1r")
            EB = 1024  # edge-encoder big: 2 psum banks, 2 mm chunks of 512
            for b in range(NPAIR // EB):
                ps = ps1p.tile([32, EB], F32, tag="ps1")
                for k in range(2):
                    sl_ps = slice(k * 512, (k + 1) * 512)
                    sl_g = slice(b * EB + k * 512, b * EB + (k + 1) * 512)
                    nc.tensor.matmul(ps[:, sl_ps], wr("eeW1"),
                                     ew_sb[:, sl_g],
                                     start=True, stop=True)
                gsl = slice(b * EB, (b + 1) * EB)
                if b % 2 == 0:
                    nc.scalar.activation(e1r[:, gsl], ps[:, :], RELU,
                                         bias=w["eeb1"])
                else:
                    nc.vector.tensor_scalar(e1r[:, gsl], ps[:, :],
                                            w["eeb1"], 0.0, ADD, MAX)
            e2r = bp.tile([32, NPAIR], BF16, tag="e2r")
            for b in range(NPAIR // EB):
                ps = ps1p.tile([32, EB], F32, tag="ps1")
                for k in range(2):
                    sl_ps = slice(k * 512, (k + 1) * 512)
                    sl_g = slice(b * EB + k * 512, b * EB + (k + 1) * 512)
                    nc.tensor.matmul(ps[:, sl_ps], wr("eeW2"),
                                     e1r[:, sl_g],
                                     start=True, stop=True)
                gsl = slice(b * EB, (b + 1) * EB)
                if b % 2 == 0:
                    nc.scalar.activation(e2r[:, gsl], ps[:, :], RELU,
                                         bias=w["eeb2"])
                else:
                    nc.vector.tensor_scalar(e2r[:, gsl], ps[:, :],
                                            w["eeb2"], 0.0, ADD, MAX)
            e2rr = e2r[:, :]

            # ---- pairwise head, per transition ----
            # big = 8 i-rows = 768 pairs in a (128,1024) psum tile: two
            # 4-row sub-chunks of 384 at bank offsets 0 and 512.  The two
            # sub-chunks' h2 outputs are K-stacked onto partitions 0-63 /
            # 64-127 so h3 is one matmul with block-diag W3 -> (2,384).
            for t in range(NT):
                stage2 = stp.tile([2, NPAIR // 2], F32, tag="stage2")
                for b in range(12):
                    i0 = b * 8
                    ps1 = ps1p.tile([128, 1024], F32, tag="ps1")
                    for k in range(2):
                        ik = i0 + 4 * k
                        po = k * 512
                        # xne[:, i] for i in [ik, ik+4), each column x96
                        a_rhs = (xer[:, (t + 1) * D + ik:(t + 1) * D + ik + 4]
                                 .unsqueeze(2).broadcast_to((128, 4, D)))
                        # xce[:, j] run over all 96 j, repeated x4
                        b_rhs = (xer[:, t * D:(t + 1) * D]
                                 .unsqueeze(1).broadcast_to((128, 4, D)))
                        nc.tensor.matmul(ps1[:, po:po + 384], wr("udW1n"),
                                         a_rhs, start=True, stop=False)
                        nc.tensor.matmul(ps1[:, po:po + 384], wr("udW1c"),
                                         b_rhs, start=False, stop=False)
                        nc.tensor.matmul(ps1[:, po:po + 384], wr("udW1e"),
                                         e2rr[:, ik * D:ik * D + 384],
                                         start=False, stop=True)
                    h1r = h1p.tile([128, 768], BF16, tag="h1")
                    nc.scalar.activation(
                        h1r[:, :].rearrange("p (k n) -> p k n", k=2),
                        ps1[:, :].rearrange("p (k n) -> p k n", k=2)[:, :, 0:384],
                        RELU, bias=w["udb1"])
                    ps2 = ps2p.tile([128, 512], F32, tag="ps2")
                    for k in range(2):
                        nc.tensor.matmul(ps2[64 * k:64 * (k + 1), 0:384],
                                         wr("udW2"),
                                         h1r[:, k * 384:k * 384 + 384],
                                         start=True, stop=True)
                    h2r = h2p.tile([128, 384], BF16, tag="h2")
                    nc.vector.tensor_scalar(h2r[:, :], ps2[:, 0:384],
                                            w["udb2s"], 0.0, ADD, MAX)
                    ps3 = ps3p.tile([2, 384], F32, tag="ps3")
                    nc.tensor.matmul(ps3[:, :], wr("udW3d"),
                                     h2r[:, :], start=True, stop=True)
                    # evacuate + bias/NEG-mask in one DVE op (masked pairs
                    # get raw + (-1e9): 1e-7 relative from exact -1e9)
                    nc.vector.tensor_tensor(stage2[:, b * 384:(b + 1) * 384],
                                            ps3[:, :],
                                            w["addm2"][:, b * 384:(b + 1) * 384],
                                            ADD)
                # row 8b+4s+r of cls[t] <- stage2[s, b*384+r*96 : +96]
                nc.sync.dma_start(
                    out=cls[t, :, :].rearrange("(b s r) j -> s b r j",
                                               b=12, s=2, r=4),
                    in_=stage2[:, :].rearrange("s (b r j) -> s b r j",
                                               b=12, r=4))

            # ---- dist head, batched over all 8 transitions ----
            dist_sb = bp.tile([1, NT * D], F32, tag="dist")
            for c in range(2):
                sl = slice(c * 384, (c + 1) * 384)
                sln = slice(D + c * 384, D + (c + 1) * 384)
                psd = ps1p.tile([128, 384], F32, tag="ps1")
                nc.tensor.matmul(psd[:, :], wr("dpW1c"), xe[:, sl],
                                 start=True, stop=False)
                nc.tensor.matmul(psd[:, :], wr("dpW1n"), xe[:, sln],
                                 start=False, stop=True)
                d1r = h1p.tile([128, 384], F32, tag="d1")
                nc.scalar.activation(d1r[:, :], psd[:, :], RELU,
                                     bias=w["dpb1"])
                psd2 = ps2p.tile([64, 384], F32, tag="ps2")
                nc.tensor.matmul(psd2[:, :], wr("dpW2"),
                                 d1r[:, :], start=True, stop=True)
                d2r = h2p.tile([64, 384], F32, tag="d2")
                nc.vector.tensor_scalar(d2r[:, :], psd2[:, :],
                                        w["dpb2"], 0.0, ADD, MAX)
                psd3 = ps3p.tile([1, 384], F32, tag="ps3")
                nc.tensor.matmul(psd3[:, :], wr("dpW3"),
                                 d2r[:, :], start=True, stop=True)
                nc.vector.tensor_scalar(dist_sb[:, sl], psd3[:, :],
                                        w["dpb3"], None, ADD)
            nc.sync.dma_start(out=dst[:, :], in_=dist_sb[:, :])

    nc.finalize()
    return nc


def _make_in_maps(inputs):
    x = np.asarray(inputs["x"], np.float32)              # (66, 128, 192)
    edge_w = np.asarray(inputs["edge_w"], np.float32)    # (192, 192)
    batch = np.asarray(inputs["batch"])
    time_i = np.asarray(inputs["time_i"])

    wvals = {
        "neW1": inputs["ne_W1"], "neb1": np.asarray(inputs["ne_b1"]).reshape(128, 1),
        "neW2": inputs["ne_W2"], "neb2": np.asarray(inputs["ne_b2"]).reshape(128, 1),
        "eeW1": inputs["ee_W1"], "eeb1": np.asarray(inputs["ee_b1"]).reshape(32, 1),
        "eeW2": inputs["ee_W2"], "eeb2": np.asarray(inputs["ee_b2"]).reshape(32, 1),
        "udW1n": inputs["ud_W1"][0:128], "udW1c": inputs["ud_W1"][128:256],
        "udW1e": inputs["ud_W1"][256:288],
        "udb1": np.asarray(inputs["ud_b1"]).reshape(128, 1),
        "udW2": inputs["ud_W2"],
        "udb2s": np.concatenate([np.asarray(inputs["ud_b2"]),
                                 np.asarray(inputs["ud_b2"])]).reshape(128, 1),
        "udW3d": _w3_blockdiag(inputs["ud_W3"]),
        "dpW1c": inputs["dp_W1"][0:128], "dpW1n": inputs["dp_W1"][128:256],
        "dpb1": np.asarray(inputs["dp_b1"]).reshape(128, 1),
        "dpW2": inputs["dp_W2"], "dpb2": np.asarray(inputs["dp_b2"]).reshape(64, 1),
        "dpW3": np.asarray(inputs["dp_W3"]).reshape(64, 1),
        "dpb3": np.asarray(inputs["dp_b3"]).reshape(1, 1),
    }
    wvals = {k: np.asarray(v, np.float32) for k, v in wvals.items()}

    def pack(table, dtype, extra):
        ncol = sum(c for _, _, c in table)
        out = np.zeros((128, ncol), dtype)
        off = 0
        for nm, nr, ncol_i in table:
            v = extra[nm] if nm in extra else wvals[nm]
            out[0:nr, off:off + ncol_i] = np.asarray(v).astype(dtype)
            off += ncol_i
        return out

    ud_b3 = float(np.asarray(inputs["ud_b3"]).reshape(-1)[0])

    in_maps = []
    for c in range(8):
        g = c // 4
        tb = (c % 4) * NT
        t0 = int(time_i[g])
        idx = np.nonzero(batch == g)[0]
        xloc = x[t0 + tb:t0 + tb + NF][:, :, idx]        # (9, 128, 96)
        xin = np.ascontiguousarray(
            xloc.transpose(1, 0, 2).reshape(H, NF * D), np.float32)
        ew_g = edge_w[idx[:, None], idx[None, :]]        # (96, 96)
        keep = ((ew_g != 0.0) | np.eye(D, dtype=bool)).astype(np.float32)
        addm = (keep * ud_b3 + (1.0 - keep) * NEG).astype(np.float32)
        # addm in the (group s, big b, row r, j) layout of stage2
        addm2 = np.ascontiguousarray(
            addm.reshape(12, 2, 4, D).transpose(1, 0, 2, 3).reshape(2, NPAIR // 2))
        m = {
            "xin": xin,
            "wpk": pack(PACK_F32, np.float32, {"addm2": addm2}),
            "wpkb": pack(PACK_BF16, ml_dtypes.bfloat16,
                         {"ewf": ew_g.reshape(1, NPAIR)}),
        }
        in_maps.append(m)
    return in_maps


def kernel(**inputs):
    if "prog" not in _prog_cache:
        _prog_cache["prog"] = _build_program()
    nc = _prog_cache["prog"]
    in_maps = _make_in_maps(inputs)
    res = run_bass_kernel_spmd(nc, in_maps, list(range(8)))
    class_out = np.empty((G, T - 1, D, D), np.float32)
    dist_out = np.empty((G, T - 1, D), np.float32)
    for c in range(8):
        g = c // 4
        tb = (c % 4) * NT
        class_out[g, tb:tb + NT] = res.results[c]["cls"]
        dist_out[g, tb:tb + NT] = res.results[c]["dst"].reshape(NT, D)
    return (class_out, dist_out)


# revision 26
# speedup vs baseline: 1.3952x; 1.3952x over previous
"""Trainium2 Bass kernel for nn_InterpNetwork (gnn_message_passing).

Sharding: 2 graphs x 32 transitions = 64 (g,t) units -> 8 cores get 8
transitions each (cores 0-3: graph 0, cores 4-7: graph 1). Weights
replicated; no collectives.

Per-core device pipeline (feature-major layouts, features on partitions):
  - node encoder: 2 matmuls over all 9 local frames at once + ACT relu.
  - edge encoder: K=1 outer-product matmul + K=32 matmul over the 9216
    (i,j) pairs, relu split between ACT and DVE.
  - pairwise update-detector layer 1 decomposed into 3 accumulating
    matmuls per 384-pair chunk: xne[:,i] / xce[:,j] broadcast via
    stride-0 access patterns on the moving operand (K=128 each) plus the
    edge-feature pass (K=32).  ACT applies bias+relu from PSUM.
  - layer 2 (128->64) matmul + DVE fused bias+relu; layer 3 (64->1)
    matmul; gpsimd SWDGE scatters the (1,384) rows into a (96,96) tile,
    DVE applies the precomputed edge masks (bias+(-1e9) fill), DMA out.
  - dist head batched over all 8 transitions at the end.
Matmuls run as float32r (1 cyc/row at N>=256) with fp32 accumulate.
"""

import ml_dtypes
import numpy as np

import concourse.bass as bass
from concourse import bacc
import concourse.mybir as mybir
import concourse.tile as tile
from concourse.bass_utils import run_bass_kernel_spmd

F32 = mybir.dt.float32
BF16 = mybir.dt.bfloat16
RELU = mybir.ActivationFunctionType.Relu
ADD = mybir.AluOpType.add
MAX = mybir.AluOpType.max

G, T, H, D = 2, 33, 128, 96
NT = 8            # transitions per core
NF = NT + 1       # frames per core
NPAIR = D * D     # 9216
NEG = -1e9

# packed-weight layouts: (name, rows, cols) -> column offsets accumulate
PACK_F32 = [
    ("neW1", 128, 128), ("neW2", 128, 128), ("dpW1c", 128, 128),
    ("dpW1n", 128, 128), ("dpW2", 128, 64), ("neb1", 128, 1),
    ("neb2", 128, 1), ("udb1", 128, 1), ("udb2s", 128, 1), ("dpb1", 128, 1),
    ("dpb2", 64, 1), ("dpW3", 64, 1), ("dpb3", 1, 1), ("eeb1", 32, 1),
    ("eeb2", 32, 1), ("addm2", 2, NPAIR // 2),
]
PACK_BF16 = [
    ("udW1n", 128, 128), ("udW1c", 128, 128), ("udW1e", 32, 128),
    ("udW2", 128, 64), ("udW3d", 128, 2), ("eeW1", 1, 32), ("eeW2", 32, 32),
    ("ewf", 1, NPAIR),
]
NCOL_F32 = sum(c for _, _, c in PACK_F32)
NCOL_BF16 = sum(c for _, _, c in PACK_BF16)

_prog_cache = {}


def _w3_blockdiag(w3):
    w3 = np.asarray(w3, np.float32).reshape(64)
    out = np.zeros((128, 2), np.float32)
    out[0:64, 0] = w3
    out[64:128, 1] = w3
    return out


def _build_program():
    nc = bacc.Bacc()

    def din(name, shape, dt=F32):
        return nc.declare_dram_parameter(name, list(shape), dt, isOutput=False)

    xin = din("xin", (H, NF * D))          # 9 frames feature-major
    wpk = din("wpk", (128, NCOL_F32))
    wpkb = din("wpkb", (128, NCOL_BF16), BF16)

    cls = nc.declare_dram_parameter("cls", [NT, D, D], F32, isOutput=True)
    dst = nc.declare_dram_parameter("dst", [1, NT * D], F32, isOutput=True)

    with tile.TileContext(nc) as tc:
        with (
            tc.tile_pool(name="w", bufs=1) as wp,
            tc.tile_pool(name="big", bufs=1) as bp,
            tc.tile_pool(name="h1", bufs=3) as h1p,
            tc.tile_pool(name="h2", bufs=3) as h2p,
            tc.tile_pool(name="st", bufs=2) as stp,
            tc.tile_pool(name="ps1", bufs=2, space="PSUM") as ps1p,
            tc.tile_pool(name="ps2", bufs=1, space="PSUM") as ps2p,
            tc.tile_pool(name="ps3", bufs=2, space="PSUM") as ps3p,
        ):
            wsb = wp.tile([128, NCOL_F32], F32, tag="wpk")
            nc.gpsimd.dma_start(out=wsb[:, :], in_=wpk[:, :])
            wsbb = wp.tile([128, NCOL_BF16], BF16, tag="wpkb")
            nc.gpsimd.dma_start(out=wsbb[:, :], in_=wpkb[:, :])
            w = {}
            off = 0
            for nm, nr, ncol in PACK_F32:
                w[nm] = wsb[0:nr, off:off + ncol]
                off += ncol
            off = 0
            for nm, nr, ncol in PACK_BF16:
                w[nm] = wsbb[0:nr, off:off + ncol]
                off += ncol

            def wr(name):
                return w[name]

            # ---- node encoder over all NF frames ----
            x_sb = bp.tile([H, NF * D], F32, tag="x")
            nc.gpsimd.dma_start(out=x_sb[:, :], in_=xin[:, :])
            xe1 = bp.tile([128, NF * D], F32, tag="xe1")
            NE_CH = 432  # 864 = 2 chunks, each >=256 so f32r runs 1cyc/row
            for c in range(2):
                sl = slice(c * NE_CH, (c + 1) * NE_CH)
                ps = ps1p.tile([128, NE_CH], F32, tag="ps1")
                nc.tensor.matmul(ps[:, :], wr("neW1"),
                                 x_sb[:, sl], start=True, stop=True)
                nc.scalar.activation(xe1[:, sl], ps[:, :], RELU, bias=w["neb1"])
            xe = bp.tile([128, NF * D], F32, tag="xe")
            xe_bf = bp.tile([128, NF * D], BF16, tag="xebf")
            for c in range(2):
                sl = slice(c * NE_CH, (c + 1) * NE_CH)
                ps = ps1p.tile([128, NE_CH], F32, tag="ps1")
                nc.tensor.matmul(ps[:, :], wr("neW2"),
                                 xe1[:, sl], start=True, stop=True)
                nc.scalar.activation(xe[:, sl], ps[:, :], RELU, bias=w["neb2"])
                nc.scalar.activation(xe_bf[:, sl], ps[:, :], RELU,
                                     bias=w["neb2"])
            xer = xe_bf[:, :]

            # ---- edge encoder over all 9216 pairs ----
            ew_sb = w["ewf"]
            e1r = bp.tile([32, NPAIR], BF16, tag="e1r")
            EB = 1024  # edge-encoder big: 2 psum banks, 2 mm chunks of 512
            for b in range(NPAIR // EB):
                ps = ps1p.tile([32, EB], F32, tag="ps1")
                for k in range(2):
                    sl_ps = slice(k * 512, (k + 1) * 512)
                    sl_g = slice(b * EB + k * 512, b * EB + (k + 1) * 512)
                    nc.tensor.matmul(ps[:, sl_ps], wr("eeW1"),
                                     ew_sb[:, sl_g],
                                     start=True, stop=True)
                gsl = slice(b * EB, (b + 1) * EB)
                if b % 2 == 0:
                    nc.scalar.activation(e1r[:, gsl], ps[:, :], RELU,
                                         bias=w["eeb1"])
                else:
                    nc.vector.tensor_scalar(e1r[:, gsl], ps[:, :],
                                            w["eeb1"], 0.0, ADD, MAX)
            e2r = bp.tile([32, NPAIR], BF16, tag="e2r")
            for b in range(NPAIR // EB):
                ps = ps1p.tile([32, EB], F32, tag="ps1")
                for k in range(2):
                    sl_ps = slice(k * 512, (k + 1) * 512)
                    sl_g = slice(b * EB + k * 512, b * EB + (k + 1) * 512)
                    nc.tensor.matmul(ps[:, sl_ps], wr("eeW2"),
                                     e1r[:, sl_g],
                                     start=True, stop=True)
                gsl = slice(b * EB, (b + 1) * EB)
                if b % 2 == 0:
                    nc.scalar.activation(e2r[:, gsl], ps[:, :], RELU,
                                         bias=w["eeb2"])
                else:
                    nc.vector.tensor_scalar(e2r[:, gsl], ps[:, :],
                                            w["eeb2"], 0.0, ADD, MAX)
            e2rr = e2r[:, :]

            # ---- pairwise head, per transition ----
            # big = 8 i-rows = 768 pairs in a (128,1024) psum tile: two
            # 4-row sub-chunks of 384 at bank offsets 0 and 512.  The two
            # sub-chunks' h2 outputs are K-stacked onto partitions 0-63 /
            # 64-127 so h3 is one matmul with block-diag W3 -> (2,384).
            for t in range(NT):
                stage2 = stp.tile([2, NPAIR // 2], F32, tag="stage2")
                for b in range(12):
                    i0 = b * 8
                    ps1 = ps1p.tile([128, 1024], F32, tag="ps1")
                    # weight-major order: each stationary loads once per big
                    for k in range(2):
                        ik = i0 + 4 * k
                        a_rhs = (xer[:, (t + 1) * D + ik:(t + 1) * D + ik + 4]
                                 .unsqueeze(2).broadcast_to((128, 4, D)))
                        nc.tensor.matmul(ps1[:, k * 512:k * 512 + 384],
                                         wr("udW1n"), a_rhs,
                                         start=True, stop=False)
                    b_rhs = (xer[:, t * D:(t + 1) * D]
                             .unsqueeze(1).broadcast_to((128, 4, D)))
                    for k in range(2):
                        nc.tensor.matmul(ps1[:, k * 512:k * 512 + 384],
                                         wr("udW1c"), b_rhs,
                                         start=False, stop=False)
                    for k in range(2):
                        ik = i0 + 4 * k
                        nc.tensor.matmul(ps1[:, k * 512:k * 512 + 384],
                                         wr("udW1e"),
                                         e2rr[:, ik * D:ik * D + 384],
                                         start=False, stop=True)
                    h1r = h1p.tile([128, 768], BF16, tag="h1")
                    nc.scalar.activation(
                        h1r[:, :].rearrange("p (k n) -> p k n", k=2),
                        ps1[:, :].rearrange("p (k n) -> p k n", k=2)[:, :, 0:384],
                        RELU, bias=w["udb1"])
                    ps2 = ps2p.tile([128, 512], F32, tag="ps2")
                    for k in range(2):
                        nc.tensor.matmul(ps2[64 * k:64 * (k + 1), 0:384],
                                         wr("udW2"),
                                         h1r[:, k * 384:k * 384 + 384],
                                         start=True, stop=True)
                    h2r = h2p.tile([128, 384], BF16, tag="h2")
                    nc.vector.tensor_scalar(h2r[:, :], ps2[:, 0:384],
                                            w["udb2s"], 0.0, ADD, MAX)
                    ps3 = ps3p.tile([2, 384], F32, tag="ps3")
                    nc.tensor.matmul(ps3[:, :], wr("udW3d"),
                                     h2r[:, :], start=True, stop=True)
                    # evacuate + bias/NEG-mask in one DVE op (masked pairs
                    # get raw + (-1e9): 1e-7 relative from exact -1e9)
                    nc.vector.tensor_tensor(stage2[:, b * 384:(b + 1) * 384],
                                            ps3[:, :],
                                            w["addm2"][:, b * 384:(b + 1) * 384],
                                            ADD)
                # row 8b+4s+r of cls[t] <- stage2[s, b*384+r*96 : +96]
                nc.sync.dma_start(
                    out=cls[t, :, :].rearrange("(b s r) j -> s b r j",
                                               b=12, s=2, r=4),
                    in_=stage2[:, :].rearrange("s (b r j) -> s b r j",
                                               b=12, r=4))

            # ---- dist head, batched over all 8 transitions ----
            dist_sb = bp.tile([1, NT * D], F32, tag="dist")
            for c in range(2):
                sl = slice(c * 384, (c + 1) * 384)
                sln = slice(D + c * 384, D + (c + 1) * 384)
                psd = ps1p.tile([128, 384], F32, tag="ps1")
                nc.tensor.matmul(psd[:, :], wr("dpW1c"), xe[:, sl],
                                 start=True, stop=False)
                nc.tensor.matmul(psd[:, :], wr("dpW1n"), xe[:, sln],
                                 start=False, stop=True)
                d1r = h1p.tile([128, 384], F32, tag="d1")
                nc.scalar.activation(d1r[:, :], psd[:, :], RELU,
                                     bias=w["dpb1"])
                psd2 = ps2p.tile([64, 384], F32, tag="ps2")
                nc.tensor.matmul(psd2[:, :], wr("dpW2"),
                                 d1r[:, :], start=True, stop=True)
                d2r = h2p.tile([64, 384], F32, tag="d2")
                nc.vector.tensor_scalar(d2r[:, :], psd2[:, :],
                                        w["dpb2"], 0.0, ADD, MAX)
                psd3 = ps3p.tile([1, 384], F32, tag="ps3")
                nc.tensor.matmul(psd3[:, :], wr("dpW3"),
                                 d2r[:, :], start=True, stop=True)
                nc.vector.tensor_scalar(dist_sb[:, sl], psd3[:, :],
                                        w["dpb3"], None, ADD)
            nc.sync.dma_start(out=dst[:, :], in_=dist_sb[:, :])

    nc.finalize()
    return nc


def _make_in_maps(inputs):
    x = np.asarray(inputs["x"], np.float32)              # (66, 128, 192)
    edge_w = np.asarray(inputs["edge_w"], np.float32)    # (192, 192)
    batch = np.asarray(inputs["batch"])
    time_i = np.asarray(inputs["time_i"])

    wvals = {
        "neW1": inputs["ne_W1"], "neb1": np.asarray(inputs["ne_b1"]).reshape(128, 1),
        "neW2": inputs["ne_W2"], "neb2": np.asarray(inputs["ne_b2"]).reshape(128, 1),
        "eeW1": inputs["ee_W1"], "eeb1": np.asarray(inputs["ee_b1"]).reshape(32, 1),
        "eeW2": inputs["ee_W2"], "eeb2": np.asarray(inputs["ee_b2"]).reshape(32, 1),
        "udW1n": inputs["ud_W1"][0:128], "udW1c": inputs["ud_W1"][128:256],
        "udW1e": inputs["ud_W1"][256:288],
        "udb1": np.asarray(inputs["ud_b1"]).reshape(128, 1),
        "udW2": inputs["ud_W2"],
        "udb2s": np.concatenate([np.asarray(inputs["ud_b2"]),
                                 np.asarray(inputs["ud_b2"])]).reshape(128, 1),
        "udW3d": _w3_blockdiag(inputs["ud_W3"]),
        "dpW1c": inputs["dp_W1"][0:128], "dpW1n": inputs["dp_W1"][128:256],
        "dpb1": np.asarray(inputs["dp_b1"]).reshape(128, 1),
        "dpW2": inputs["dp_W2"], "dpb2": np.asarray(inputs["dp_b2"]).reshape(64, 1),
        "dpW3": np.asarray(inputs["dp_W3"]).reshape(64, 1),
        "dpb3": np.asarray(inputs["dp_b3"]).reshape(1, 1),
    }
    wvals = {k: np.asarray(v, np.float32) for k, v in wvals.items()}

    def pack(table, dtype, extra):
        ncol = sum(c for _, _, c in table)
        out = np.zeros((128, ncol), dtype)
        off = 0
        for nm, nr, ncol_i in table:
            v = extra[nm] if nm in extra else wvals[nm]
            out[0:nr, off:off + ncol_i] = np.asarray(v).astype(dtype)
            off += ncol_i
        return out

    ud_b3 = float(np.asarray(inputs["ud_b3"]).reshape(-1)[0])

    in_maps = []
    for c in range(8):
        g = c // 4
        tb = (c % 4) * NT
        t0 = int(time_i[g])
        idx = np.nonzero(batch == g)[0]
        xloc = x[t0 + tb:t0 + tb + NF][:, :, idx]        # (9, 128, 96)
        xin = np.ascontiguousarray(
            xloc.transpose(1, 0, 2).reshape(H, NF * D), np.float32)
        ew_g = edge_w[idx[:, None], idx[None, :]]        # (96, 96)
        keep = ((ew_g != 0.0) | np.eye(D, dtype=bool)).astype(np.float32)
        addm = (keep * ud_b3 + (1.0 - keep) * NEG).astype(np.float32)
        # addm in the (group s, big b, row r, j) layout of stage2
        addm2 = np.ascontiguousarray(
            addm.reshape(12, 2, 4, D).transpose(1, 0, 2, 3).reshape(2, NPAIR // 2))
        m = {
            "xin": xin,
            "wpk": pack(PACK_F32, np.float32, {"addm2": addm2}),
            "wpkb": pack(PACK_BF16, ml_dtypes.bfloat16,
                         {"ewf": ew_g.reshape(1, NPAIR)}),
        }
        in_maps.append(m)
    return in_maps


def kernel(**inputs):
    if "prog" not in _prog_cache:
        _prog_cache["prog"] = _build_program()
    nc = _prog_cache["prog"]
    in_maps = _make_in_maps(inputs)
    res = run_bass_kernel_spmd(nc, in_maps, list(range(8)))
    class_out = np.empty((G, T - 1, D, D), np.float32)
    dist_out = np.empty((G, T - 1, D), np.float32)
    for c in range(8):
        g = c // 4
        tb = (c % 4) * NT
        class_out[g, tb:tb + NT] = res.results[c]["cls"]
        dist_out[g, tb:tb + NT] = res.results[c]["dst"].reshape(NT, D)
    return (class_out, dist_out)
